# revision 42
# baseline (speedup 1.0000x reference)
"""Trainium2 Bass kernel for AssignmentWeightedAverage (nms_detection).

cost[m, n] = 0.4*(1 - box_iou) + 0.3*(1 - mask_iou) + 0.3*euclid(feat)

The heavy part is mask_iou's intersection matrix: a [256, 256] Gram matrix
over 256x(480*854) boolean masks (~105 MB each).  Strategy: shard the
CONTRACTION (pixel) axis across the 8 cores -- each core reads 1/8 of both
masks (~26 MB), computes a partial intersection Gram + partial areas, then
a ReduceScatter hands each core a 32-column stripe of the summed partials;
each core finishes the tiny box-iou / reid / combine math for its stripe
and the host concatenates the 8 stripes.

Key tricks:
- masks stay RAW 0/1 bytes, declared fp8e4: 0x01 is the subnormal 2^-9, so
  matmul products are exactly 2^-18 and the f32 PSUM accumulation is exact;
  one 2^18 rescale during PSUM evacuation recovers exact counts.  No host
  value conversion, no DMA cast (1 byte/elem HBM + SBUF traffic).
- track-side mask areas ride along as a ones-column in the rhs (area1).
- current-side areas: the DVE taps the same SBUF bytes bitcast as u16 and
  integer-adds tile-blocks at 2x rate; byte sums never overflow; one
  and/sub extract + fp16 pack + a tiny ones-matmul gives area2.
"""

import numpy as np
import ml_dtypes

from concourse import bass, bacc, mybir, tile
from concourse.bass_utils import run_bass_kernel_spmd

N1 = 256
N2 = 256
HW = 480 * 854            # 409920
D = 512
NCORES = 8
KPC = HW // NCORES        # 51240 pixels per core
TPC = (KPC + 127) // 128  # 401 K-tiles of 128 (last padded)
KP = TPC * 128            # 51328
M2T = 272                 # per-tile rhs width: 256 data + ones + pad (16-mult)
M2H = M2T // 2            # 136 u16 lanes per tile
CH = 48                   # max K-tiles per DMA chunk
AW = 32                   # accq accumulator width in K-tiles (power of 2)
SIZES = [8, 16, 24] + [CH] * 7 + [17]  # chunk tile counts (sum = TPC)
MT = 256 + M2T            # bytes per tile in the merged chunk layout
NS = 32                   # output columns per core stripe
SH = 256 * NS + 256 + NS  # ReduceScatter shard: inter[256,32] + area1 + area2
W_BOX, W_MASK, W_REID = 0.4, 0.3, 0.3
RESCALE = float(2 ** 18)  # undo the fp8-subnormal 2^-18 product scale

f16 = mybir.dt.float16
f32 = mybir.dt.float32
bf16 = mybir.dt.bfloat16
u16 = mybir.dt.uint16
f8 = mybir.dt.float8e4
COPY = mybir.ActivationFunctionType.Copy
A = mybir.AluOpType
DR = mybir.MatmulPerfMode.DoubleRow

_CACHE = {}


def _build():
    if "nc" in _CACHE:
        return _CACHE["nc"]
    nc = bacc.Bacc("TRN2", target_bir_lowering=False, debug=False,
                   num_devices=NCORES)
    mdd = nc.dram_tensor("md", [128, TPC * MT], f8, kind="ExternalInput")
    tftd = nc.dram_tensor("tft", [D, N1], f32, kind="ExternalInput")
    cftd = nc.dram_tensor("cft", [D, N2], f32, kind="ExternalInput")
    tbd = nc.dram_tensor("tb", [N1, 4], f32, kind="ExternalInput")
    cbtd = nc.dram_tensor("cbt", [4, N2], f32, kind="ExternalInput")
    outd = nc.dram_tensor("out", [N1, N2], f32, kind="ExternalOutput")

    # small leading chunks so the PE starts early; 96-tile steady state
    assert sum(SIZES) == TPC
    chunks = []
    s = 0
    for c in SIZES:
        chunks.append((s, c))
        s += c

    with tile.TileContext(nc) as tc:
        with tc.tile_pool(name="pm1", bufs=4) as pm1, \
             tc.tile_pool(name="pone", bufs=1) as pone, \
             tc.tile_pool(name="pmisc", bufs=1) as pmisc, \
             tc.tile_pool(name="pwork", bufs=2) as pwork, \
             tc.tile_pool(name="pps", bufs=1, space="PSUM") as pps, \
             tc.tile_pool(name="psc", bufs=3, space="PSUM") as psc, \
             tc.tile_pool(name="pdram", bufs=1, space="DRAM") as pdram:

            # dummy collective first: absorbs the first-trigger ncfw warmup
            # so the real ReduceScatter starts with ~1us delay instead of ~12
            dmy_in = pdram.tile([64], f32, tag="dmy_in")
            dmy_out = pdram.tile([64], f32, tag="dmy_out")
            nc.gpsimd.collective_compute(
                "AllReduce", A.add, replica_groups=[list(range(NCORES))],
                ins=[dmy_in[:].opt()], outs=[dmy_out[:].opt()])

            accq = pmisc.tile([128, AW * M2H], u16, tag="accq")
            ones16_d = nc.inline_tensor(np.ones((128, 1), np.float16),
                                        name="ones16_d")
            ones16 = pone.tile([128, 1], f16, tag="ones16")
            onesb_d = nc.inline_tensor(np.ones((128, 1), ml_dtypes.bfloat16),
                                       name="onesb_d")
            onesb = pone.tile([128, 1], bf16, tag="onesb")
            eye_d = nc.inline_tensor(np.eye(128, dtype=np.float32), name="eye_d")
            eye = pone.tile([128, 128], f32, tag="eye")

            ps0 = pps.tile([128, 257], f32, tag="ps0")
            ps1 = pps.tile([128, 257], f32, tag="ps1")

            # ---- mask Gram loop (the heavy part) ----
            # one merged [m1-block | m2-block] DMA per chunk, rings alternate
            init = 0
            d_t1c1 = None
            for ci, (s0, cnt) in enumerate(chunks):
                ring = nc.sync if ci % 2 == 0 else nc.scalar
                td = pm1.tile([128, cnt * MT], f8, tag="td")
                d_t1 = ring.dma_start(td[:], mdd[:, s0 * MT:(s0 + cnt) * MT])
                t1 = td[:, 0:cnt * 256]
                t2 = td[:, cnt * 256:cnt * MT]
                if ci == 0:
                    # constants load behind chunk 0 on the HWDGE rings
                    nc.sync.dma_start(ones16[:], ones16_d[:])
                    nc.sync.dma_start(onesb[:], onesb_d[:])
                    nc.scalar.dma_start(eye[:], eye_d[:])
                if ci == 1:
                    d_t1c1 = d_t1
                # fp8 DoubleRow: two K-tiles per matmul instruction
                t = 0
                while t < cnt:
                    g = s0 + t
                    if t + 1 < cnt:
                        l3 = t1[:, t * 256:(t + 2) * 256].rearrange(
                            "p (j m) -> p j m", j=2)
                        r3 = t2[:, t * M2T:(t + 2) * M2T].rearrange(
                            "p (j w) -> p j w", j=2)[:, :, 0:257]
                        nc.tensor.matmul(ps0[:], l3[:, :, 0:128], r3,
                                         perf_mode=DR,
                                         start=(g == 0), stop=False)
                        nc.tensor.matmul(ps1[:], l3[:, :, 128:256], r3,
                                         perf_mode=DR,
                                         start=(g == 0), stop=False)
                        t += 2
                    else:
                        lc = t * 256
                        rhs = t2[:, t * M2T:t * M2T + 257]
                        nc.tensor.matmul(ps0[:], t1[:, lc:lc + 128], rhs,
                                         start=(g == 0), stop=(g == TPC - 1))
                        nc.tensor.matmul(ps1[:], t1[:, lc + 128:lc + 256], rhs,
                                         start=(g == 0), stop=(g == TPC - 1))
                        t += 1
                # DVE tap for area2: u16 integer adds over the same bytes
                for off in range(0, cnt, AW):
                    w = min(AW, cnt - off)
                    a = min(w, init)
                    if a > 0:
                        nc.vector.tensor_add(
                            accq[:, :a * M2H], accq[:, :a * M2H],
                            t2[:, off * M2T:(off + a) * M2T].bitcast(u16))
                    if w > init:
                        nc.vector.tensor_copy(
                            accq[:, init * M2H:w * M2H],
                            t2[:, (off + init) * M2T:(off + w) * M2T].bitcast(u16))
                        init = w

            # fold accq's AW tile-blocks down to 2 (byte sums stay <= 255)
            cur = AW
            while cur > 2:
                if cur % 2:
                    nc.vector.tensor_add(
                        accq[:, :M2H], accq[:, :M2H],
                        accq[:, (cur - 1) * M2H:cur * M2H])
                    cur -= 1
                    if cur == 2:
                        break
                h = cur // 2
                nc.vector.tensor_add(accq[:, :h * M2H], accq[:, :h * M2H],
                                     accq[:, h * M2H:2 * h * M2H])
                cur = h
            # extract byte lanes: lo = even pixels' sums, hi = 256*odd sums
            lo2 = pmisc.tile([128, 2 * M2H], u16, tag="lo2")
            nc.vector.tensor_scalar(lo2[:], accq[:, :2 * M2H], 0x00FF, None,
                                    op0=A.bitwise_and)
            hi2 = pmisc.tile([128, 2 * M2H], u16, tag="hi2")
            nc.vector.tensor_sub(hi2[:], accq[:, :2 * M2H], lo2[:])
            af = pmisc.tile([128, 2 * M2T], f16, tag="af")
            nc.scalar.activation(af[:, 0:2 * M2H], lo2[:], COPY, scale=1.0)
            nc.scalar.activation(af[:, 2 * M2H:4 * M2H], hi2[:], COPY,
                                 scale=1.0 / 256.0)
            nc.vector.tensor_add(af[:, 0:M2H], af[:, 0:M2H],
                                 af[:, M2H:2 * M2H])
            nc.vector.tensor_add(af[:, M2H:2 * M2H], af[:, 2 * M2H:3 * M2H],
                                 af[:, 3 * M2H:4 * M2H])
            # af[:, 0:130] = even-pixel counts, af[:, 130:260] = odd
            psA2 = psc.tile([1, 256], f32, tag="scratch")
            rhsA2 = af[:, 0:2 * M2H].rearrange("p (a b) -> p a b", a=2)[:, :, 0:128]
            nc.tensor.matmul(psA2[:], ones16[:], rhsA2, start=True, stop=True)

            # ---- evacuate partials (rescaled 2^18) ----
            cc_sb = pmisc.tile([128, 514], f32, tag="cc_sb")
            nc.scalar.activation(cc_sb[:, 0:257], ps0[:], COPY, scale=RESCALE)
            nc.scalar.activation(cc_sb[:, 257:514], ps1[:], COPY, scale=RESCALE)
            # area1 as a [1,256] row (PE transpose of the ones-columns)
            psT0 = psc.tile([1, 128], f32, tag="scratch")
            nc.tensor.transpose(psT0[:], cc_sb[:, 256:257], eye[:])
            psT1 = psc.tile([1, 128], f32, tag="scratch")
            nc.tensor.transpose(psT1[:], cc_sb[:, 513:514], eye[:])
            arow = pmisc.tile([1, 512], f32, tag="arow")
            nc.scalar.copy(arow[:, 0:128], psT0[:])
            nc.scalar.copy(arow[:, 128:256], psT1[:])
            # area2 packed [even|odd] -> natural order row
            nc.vector.tensor_copy(
                arow[:, 256:512],
                psA2[:].rearrange("p (s q) -> p q s", s=2))

            # ---- scatter partials into shard layout and ReduceScatter ----
            # shard r (f32): [0:8192]  inter[m, 32r:32r+32] m-major
            #                [8192:8448] area1[all m] (replicated)
            #                [8448:8480] area2[32r:32r+32]
            cc_in = pdram.tile([NCORES * SH], f32, tag="cc_in")
            rs_out = pdram.tile([SH], f32, tag="rs_out")
            X = cc_in[:].rearrange("(r q) -> r q", r=NCORES)
            for h in range(2):
                dst = X[:, h * 4096:(h + 1) * 4096].rearrange(
                    "r (p j) -> p r j", j=NS)
                src = cc_sb[:, h * 257:h * 257 + 256].rearrange(
                    "p (r j) -> p r j", r=NCORES)
                nc.sync.dma_start(dst, src)
            # area1: one DMA with a step-0 (replicating) source dim
            a1src = arow[:, 0:256].rearrange(
                "p (x w) -> p x w", x=1).broadcast_to((1, NCORES, 256))
            nc.sync.dma_start(X[:, 8192:8448], a1src)
            a2src = arow[:, 256:512].rearrange("p (r j) -> p r j", r=NCORES)
            nc.scalar.dma_start(X[:, 8448:8480], a2src)
            nc.gpsimd.collective_compute(
                "ReduceScatter", A.add,
                replica_groups=[list(range(NCORES))],
                ins=[cc_in[:].opt()], outs=[rs_out[:].opt()],
            )

            # ---- local stripe work that overlaps the collective ----
            rvv = nc.vector.partition_id()
            r32v = rvv * NS
            # features: Gram + norms in bf16 (cast during DMA); defer these
            # DMAs behind the chunk-1 mask load so they don't steal SDMA
            # bandwidth from the kernel-critical first chunks
            from concourse.tile import add_dep_helper
            tf_sb = pmisc.tile([128, 4, N1], bf16, tag="tf_sb")
            dtf = nc.gpsimd.dma_start(
                tf_sb[:], tftd[:].rearrange("(i p) n -> p i n", p=128))
            cf_sb = pmisc.tile([128, 4, N2], bf16, tag="cf_sb")
            dcf = nc.gpsimd.dma_start(
                cf_sb[:], cftd[:].rearrange("(i p) n -> p i n", p=128))
            if d_t1c1 is not None:
                add_dep_helper(dtf.ins, d_t1c1.ins, sync=True,
                               reason="defer feat dma")
                add_dep_helper(dcf.ins, d_t1c1.ins, sync=True,
                               reason="defer feat dma")
            # one bank holds both Gram halves; groups are serialized so the
            # second start=True only clears has_written bits of a DONE group
            psG = pps.tile([128, 2 * N2], f32, tag="psG")
            psG0 = psG[:, 0:N2]
            psG1 = psG[:, N2:2 * N2]
            for i in range(4):
                nc.tensor.matmul(psG0, tf_sb[:, i, 0:128], cf_sb[:, i, :],
                                 start=(i == 0), stop=(i == 3))
            for i in range(4):
                nc.tensor.matmul(psG1, tf_sb[:, i, 128:256], cf_sb[:, i, :],
                                 start=(i == 0), stop=(i == 3))
            sqt = pmisc.tile([128, 4, N1], bf16, tag="sqt")
            nc.scalar.square(sqt[:], tf_sb[:])
            sqc = pmisc.tile([128, 4, N2], bf16, tag="sqc")
            nc.scalar.square(sqc[:], cf_sb[:])
            psN1 = pps.tile([128, 2], f32, tag="psN1")
            psN1h0 = psN1[:, 0:1]
            psN1h1 = psN1[:, 1:2]
            psN2 = pps.tile([1, N2], f32, tag="psN2")
            for i in range(4):
                nc.tensor.matmul(psN1h0, sqt[:, i, 0:128], onesb[:],
                                 start=(i == 0), stop=(i == 3))
            for i in range(4):
                nc.tensor.matmul(psN1h1, sqt[:, i, 128:256], onesb[:],
                                 start=(i == 0), stop=(i == 3))
            for i in range(4):
                nc.tensor.matmul(psN2[:], onesb[:], sqc[:, i, :],
                                 start=(i == 0), stop=(i == 3))

            # boxes
            tb_sb = pmisc.tile([128, 2, 4], f32, tag="tb_sb")
            nc.sync.dma_start(tb_sb[:], tbd[:].rearrange("(h p) c -> p h c", p=128))
            stage = pmisc.tile([1, 6 * 256], f32, tag="stage")
            for i in range(4):
                nc.sync.dma_start(stage[0:1, i * 256:(i + 1) * 256],
                                  cbtd[i:i + 1, :])
            tmpc = pmisc.tile([1, 256], f32, tag="tmpc")
            nc.vector.tensor_sub(stage[:, 1024:1280], stage[:, 512:768],
                                 stage[:, 0:256])
            nc.vector.tensor_sub(tmpc[:], stage[:, 768:1024], stage[:, 256:512])
            nc.vector.tensor_mul(stage[:, 1024:1280], stage[:, 1024:1280],
                                 tmpc[:])
            nc.scalar.copy(stage[0:1, 1280:1536], psN2[:])
            bc = pmisc.tile([128, 6 * 256], f32, tag="bc")
            nc.gpsimd.partition_broadcast(bc[:], stage[0:1, :])

            def bcs(c):  # this core's n-stripe of broadcast row c
                return bc[:, bass.ds(r32v + c * 256, NS)]

            # box iou + reid for this core's stripe (no collective needed)
            pred2 = pwork.tile([128, 2, NS], f32, tag="pred2")
            fin2 = pwork.tile([128, 2, NS], f32, tag="fin2")
            for h in range(2):
                tbh = tb_sb[:, h, :]
                tx1, ty1 = tbh[:, 0:1], tbh[:, 1:2]
                tx2, ty2 = tbh[:, 2:3], tbh[:, 3:4]
                wx = pwork.tile([128, NS], f32, tag="wx")
                wy = pwork.tile([128, NS], f32, tag="wy")
                t0 = pwork.tile([128, NS], f32, tag="t0")
                nc.vector.tensor_scalar(wx[:], bcs(2), tx2, None, op0=A.min)
                nc.vector.tensor_scalar(t0[:], bcs(0), tx1, None, op0=A.max)
                nc.vector.tensor_sub(wx[:], wx[:], t0[:])
                nc.vector.tensor_scalar(wx[:], wx[:], 0.0, None, op0=A.max)
                nc.vector.tensor_scalar(wy[:], bcs(3), ty2, None, op0=A.min)
                nc.vector.tensor_scalar(t0[:], bcs(1), ty1, None, op0=A.max)
                nc.vector.tensor_sub(wy[:], wy[:], t0[:])
                nc.vector.tensor_scalar(wy[:], wy[:], 0.0, None, op0=A.max)
                ib = pwork.tile([128, NS], f32, tag="ib")
                nc.vector.tensor_mul(ib[:], wx[:], wy[:])
                td1 = pwork.tile([128, 1], f32, tag="td1")
                td2 = pwork.tile([128, 1], f32, tag="td2")
                nc.vector.tensor_scalar(td1[:], tx2, tx1, None, op0=A.subtract)
                nc.vector.tensor_scalar(td2[:], ty2, ty1, None, op0=A.subtract)
                nc.vector.tensor_mul(td1[:], td1[:], td2[:])
                ub = pwork.tile([128, NS], f32, tag="ub")
                nc.vector.scalar_tensor_tensor(ub[:], bcs(4), td1[:], ib[:],
                                               op0=A.add, op1=A.subtract)
                nc.vector.reciprocal(ub[:], ub[:])
                biou = pwork.tile([128, NS], f32, tag="biou")
                nc.vector.tensor_mul(biou[:], ib[:], ub[:])
                nc.vector.tensor_scalar(pred2[:, h, :], ib[:], 0.0, None,
                                        op0=A.is_gt)
                # reid euclid
                psGh = psG0 if h == 0 else psG1
                psN1h = psN1h0 if h == 0 else psN1h1
                sq = pwork.tile([128, NS], f32, tag="sq")
                nc.vector.scalar_tensor_tensor(
                    sq[:], psG[:, bass.ds(r32v + h * N2, NS)], -2.0, bcs(5),
                    op0=A.mult, op1=A.add)
                nc.vector.tensor_scalar(sq[:], sq[:], psN1h, 0.0,
                                        op0=A.add, op1=A.max)
                reid = pwork.tile([128, NS], f32, tag="reid")
                nc.scalar.sqrt(reid[:], sq[:])
                fin = fin2[:, h, :]
                nc.vector.tensor_scalar(fin, biou[:], -W_BOX, W_BOX + W_MASK,
                                        op0=A.mult, op1=A.add)
                nc.vector.scalar_tensor_tensor(fin, reid[:], W_REID, fin,
                                               op0=A.mult, op1=A.add)

            # ---- read back the ReduceScatter shard and finish the stripe ----
            Y = rs_out[:]
            i32 = pmisc.tile([128, 2, NS], f32, tag="i32")
            nc.sync.dma_start(i32[:],
                              Y[0:8192].rearrange("(h p j) -> p h j", h=2, j=NS))
            i32h0 = i32[:, 0, :]
            i32h1 = i32[:, 1, :]
            ar = pmisc.tile([1, 288], f32, tag="ar")
            nc.scalar.dma_start(ar[:], Y[8192:8480].rearrange("(p q) -> p q", p=1))
            a1r = ar[:, 0:256]
            a2b = pmisc.tile([128, NS], f32, tag="a2b")
            nc.gpsimd.partition_broadcast(a2b[:], ar[:, 256:288])
            # area1 row back to per-partition columns (PE transpose)
            psBp = psc.tile([128, 2], f32, tag="scratch")
            nc.tensor.transpose(psBp[:, 0:1], a1r[:, 0:128], eye[0:1, 0:1])
            nc.tensor.transpose(psBp[:, 1:2], a1r[:, 128:256], eye[0:1, 0:1])

            # both halves at once: aa = area1[m] + area2[n]
            aa = pwork.tile([128, 2, NS], f32, tag="aa")
            for h in range(2):
                nc.vector.tensor_scalar(aa[:, h, :], a2b[:], psBp[:, h:h + 1],
                                        None, op0=A.add)
            interp = pwork.tile([128, 2, NS], f32, tag="interp")
            nc.vector.tensor_mul(interp[:], i32[:], pred2[:])
            um = pwork.tile([128, 2, NS], f32, tag="um")
            nc.vector.tensor_sub(um[:], aa[:], interp[:])
            nc.vector.reciprocal(um[:], um[:])
            nc.vector.tensor_mul(interp[:], interp[:], um[:])
            nc.vector.scalar_tensor_tensor(fin2[:], interp[:], -W_MASK, fin2[:],
                                           op0=A.mult, op1=A.add)
            rvs = nc.sync.partition_id()
            r32s = rvs * NS
            nc.sync.dma_start(
                outd[:].rearrange("(h p) n -> p h n", h=2)[:, :, bass.ds(r32s, NS)],
                fin2[:])

    nc.compile()
    _CACHE["nc"] = nc
    return nc


def _prep_mask_t(mask_u8, ones_col):
    """[256, HW] uint8 -> [8, 128, TPC, w] per-core transposed tile layout."""
    w = M2T if ones_col else 256
    out = np.zeros((NCORES, 128, TPC, w), dtype=np.uint8)
    if ones_col:
        out[..., 256] = 1
    for c in range(NCORES):
        chunk = mask_u8[:, c * KPC:(c + 1) * KPC]          # [256, 51240]
        ct = np.zeros((KP, N1), dtype=np.uint8)
        ct[:KPC] = chunk.T                                  # [51328, 256]
        ct = ct.reshape(TPC, 128, N1).transpose(1, 0, 2)    # [128, TPC, 256]
        out[c, :, :, :256] = ct
    return out


def kernel(track_features, current_features, track_boxes, current_boxes,
           track_time, current_time, track_masks, current_masks):
    tm = np.asarray(track_masks).reshape(N1, HW).astype(np.uint8, copy=False)
    cm = np.asarray(current_masks).reshape(N2, HW).astype(np.uint8, copy=False)
    m1 = _prep_mask_t(tm, ones_col=False)       # [8, 128, TPC, 256]
    m2 = _prep_mask_t(cm, ones_col=True)        # [8, 128, TPC, M2T]
    # merge chunk-wise: per chunk block = [m1 tiles | m2 tiles]
    md = np.empty((NCORES, 128, TPC * MT), dtype=np.uint8)
    s = 0
    off = 0
    for cnt in SIZES:
        w1 = cnt * 256
        w2 = cnt * M2T
        md[:, :, off:off + w1] = m1[:, :, s:s + cnt].reshape(NCORES, 128, w1)
        md[:, :, off + w1:off + w1 + w2] = m2[:, :, s:s + cnt].reshape(
            NCORES, 128, w2)
        s += cnt
        off += w1 + w2
    md = md.view(ml_dtypes.float8_e4m3)

    tft = np.ascontiguousarray(np.asarray(track_features, dtype=np.float32).T)
    cft = np.ascontiguousarray(np.asarray(current_features, dtype=np.float32).T)
    tb = np.ascontiguousarray(np.asarray(track_boxes, dtype=np.float32))
    cbt = np.ascontiguousarray(np.asarray(current_boxes, dtype=np.float32).T)

    in_maps = [
        {"md": md[c], "tft": tft, "cft": cft, "tb": tb, "cbt": cbt}
        for c in range(NCORES)
    ]
    nc = _build()
    res = run_bass_kernel_spmd(nc, in_maps, core_ids=list(range(NCORES)),
                               trace=_CACHE.get("trace", False))
    _CACHE["last_exec_time_ns"] = res.exec_time_ns
    out = np.empty((N1, N2), dtype=np.float32)
    for c in range(NCORES):
        out[:, c * NS:(c + 1) * NS] = np.asarray(
            res.results[c]["out"])[:, c * NS:(c + 1) * NS]
    return out
